# revision 10
# baseline (speedup 1.0000x reference)
"""CGC (Customized Gate Control) MoE kernel for Trainium2, 8 NeuronCores.

Problem: 3 inputs x_{shared,task1,task2} [4096, 1024]; three expert groups
(sh/t1/t2) of 4 experts each; expert = fc2(relu(fc1(x))) with
fc1: 1024->2048, fc2: 2048->512; three softmax gates; outputs
(out_sh, out1, out2) each [4096, 512] as gate-weighted sums of expert
outputs.

Sharding: data-parallel over batch across 8 cores (512 rows/core), all
weights replicated. No collectives.

v2 (bf16): all matmuls in bfloat16 — measured 1.16x faster per matmul
than float32r on this HW and half the HBM traffic; accuracy ~4e-3
(gate 2e-2). x is pre-transposed on the host so no PE transposes; gate
logits are computed as 24 N=512 matmuls (instead of ~240 N=8..12 ones)
with softmax done unnormalized-exp -> PE transpose -> DVE normalize
(logits are O(1), no max subtraction needed); fc2 bias via one
ones-outer-product matmul per expert, added during the DVE drain.

Per-core dataflow (batch tile b=512, partition tiles of 128):
  - xT [128, IT, 512] bf16 DMA'd directly (host pre-transposed)
  - gates: logitsT[gw,b] = sum_it wg[:,it,:].T @ xT[:,it,:] (PE),
    ex = exp(logitsT + bg) (ACT), per-bt PE transpose, DVE normalize
  - per expert e: ph[ht] = W1[:,ht].T @ xT (PE, 8 accum), relu+b1 ->
    hT bf16 (DVE/ACT alternating); o[bt] += hT[:,bt].T @ W2[ht] over ht
    (PE, PSUM accum); drain o_sb = psum + b2rep (DVE tensor_add);
    acc[head][bt] (+)= g[head][:,e] * o_sb (DVE)
  - store acc -> outputs.

fc1/fc2 are software-pipelined by SKEW=2 ht-steps so the relu epilogue
latency hides under the next fc1 block (PE queue is in-order).
"""
import sys
from contextlib import nullcontext

if "/opt/trn_rl_repo" not in sys.path:
    sys.path.insert(0, "/opt/trn_rl_repo")

import numpy as np

import concourse.bass as bass
import concourse.mybir as mybir
from concourse import bacc
from concourse.tile import TileContext
from concourse.masks import make_identity

B, I, H, O = 4096, 1024, 2048, 512
E = 4                      # experts per group
N_CORES = 8
BL = B // N_CORES          # 512 rows per core
BT = BL // 128             # 4 batch tiles
IT = I // 128              # 8 input tiles
HT = H // 128              # 16 hidden tiles
HTG = 4                    # ht-tiles per 512-wide W1 column / W2 row block
NHTG = HT // HTG           # 4 blocks

F32 = mybir.dt.float32
BF16 = mybir.dt.bfloat16
F32R = mybir.dt.float32r

GROUPS = ("t1", "t2", "sh")
GATE_W = {"sh": 2 * E + E, "t1": E + E, "t2": E + E}  # 12, 8, 8
HEADS = ("osh", "o1", "o2")


# (group, e) -> list of (head, gate_name, gate_col)
def _contribs(grp, e):
    if grp == "t1":
        return [("o1", "t1", e), ("osh", "sh", e)]
    if grp == "t2":
        return [("o2", "t2", e), ("osh", "sh", E + e)]
    return [("o1", "t1", E + e), ("o2", "t2", E + e), ("osh", "sh", 2 * E + e)]


def build_nc(loop_reps=None, mode="full", mm="bf16", tail="dve"):
    """Build the per-core kernel. loop_reps wraps the whole body in a
    hardware For_i loop (timing); mode "dma" = loads only, "compute" =
    stub weight loads (diagnostics). mm selects matmul dtype
    ("bf16"/"f32r") for A/B testing. tail is accepted for compat; the
    expert tail always runs on DVE (Pool/gpsimd cannot read PSUM nor
    run TensorScalarPtr on core-v3)."""
    DT = BF16 if mm == "bf16" else F32R
    nc = bacc.Bacc(None)

    # ---- DRAM parameters (host-prearranged layouts) -------------------
    xT = {g: nc.declare_dram_parameter(f"xT_{g}", [128, IT, BL], DT,
                                       isOutput=False) for g in GROUPS}
    w1 = {g: nc.declare_dram_parameter(f"w1_{g}", [E, NHTG, 128, IT, 512],
                                       DT, isOutput=False) for g in GROUPS}
    b1 = {g: nc.declare_dram_parameter(f"b1_{g}", [E, 128, HT], F32,
                                       isOutput=False) for g in GROUPS}
    w2 = {g: nc.declare_dram_parameter(f"w2_{g}", [E, NHTG, 128, HTG, O],
                                       DT, isOutput=False) for g in GROUPS}
    b2 = {g: nc.declare_dram_parameter(f"b2_{g}", [E, O], DT,
                                       isOutput=False) for g in GROUPS}
    wg = {g: nc.declare_dram_parameter(f"wg_{g}", [128, IT, GATE_W[g]], DT,
                                       isOutput=False) for g in GROUPS}
    bg = {g: nc.declare_dram_parameter(f"bg_{g}", [GATE_W[g]], F32,
                                       isOutput=False) for g in GROUPS}
    outs = {h: nc.declare_dram_parameter(h, [BL, O], F32, isOutput=True)
            for h in HEADS}

    with TileContext(nc) as tc:
        with tc.tile_pool(name="persist", bufs=1) as pp, \
             tc.tile_pool(name="work", bufs=1) as pw, \
             tc.tile_pool(name="ps", bufs=1, space="PSUM") as ps:
            ident = pp.tile([128, 128], F32, name="ident")
            make_identity(nc, ident[:, :])
            ones_f = pp.tile([1, 128], F32, name="ones_f")
            nc.gpsimd.memset(ones_f[:, :], 1.0)
            ones = pp.tile([1, 128], DT, name="ones")
            nc.vector.tensor_copy(ones[:, :], ones_f[:, :])

            loop_cm = tc.For_i(0, loop_reps, 1) if loop_reps else nullcontext()
            with loop_cm:
                # ---- Phase A: input loads + gates ---------------------
                xTs, gsb = {}, {}
                for g in GROUPS:
                    xTs[g] = pw.tile([128, IT, BL], DT, tag=f"xT{g}", bufs=2,
                                     name=f"xT_{g}_sb")
                    nc.sync.dma_start(xTs[g][:, :, :], xT[g][:, :, :])
                    wg_sb = pw.tile([128, IT, GATE_W[g]], DT, tag=f"wg{g}",
                                    bufs=2, name=f"wg_{g}_sb")
                    nc.sync.dma_start(wg_sb[:, :, :], wg[g][:, :, :])
                    bg_sb = pw.tile([GATE_W[g], 1], F32, tag=f"bg{g}", bufs=2,
                                    name=f"bg_{g}_sb")
                    nc.sync.dma_start(bg_sb[:, :], bg[g][:, None])
                    if mode == "dma":
                        continue
                    # logitsT [GW, BL] = sum_it wg_sb[:,it,:].T @ xT[:,it,:]
                    gps = ps.tile([GATE_W[g], BL], F32, tag="ph", bufs=4,
                                  name=f"gps_{g}")
                    for it in range(IT):
                        nc.tensor.matmul(gps[:, :], wg_sb[:, it, :],
                                         xTs[g][:, it, :],
                                         start=(it == 0), stop=(it == IT - 1))
                    # exp(logits + bg); |logits| <= ~3 so no max needed
                    ex = pw.tile([GATE_W[g], BL], F32, tag=f"ex{g}", bufs=2,
                                 name=f"ex_{g}")
                    nc.scalar.activation(ex[:, :], gps[:, :],
                                         mybir.ActivationFunctionType.Exp,
                                         bias=bg_sb[:, :], scale=1.0)
                    gsb[g] = pw.tile([128, BT, GATE_W[g]], F32, tag=f"g{g}",
                                     bufs=2, name=f"g_{g}")
                    for bt in range(BT):
                        pt = ps.tile([128, GATE_W[g]], F32, tag="ph", bufs=4,
                                     name=f"pt_{g}_{bt}")
                        nc.tensor.transpose(
                            pt[:, :], ex[:, bt * 128:(bt + 1) * 128],
                            ident[:GATE_W[g], :GATE_W[g]])
                        sm = pw.tile([128, 1], F32, tag="sm", bufs=2,
                                     name=f"sm_{g}_{bt}")
                        nc.vector.reduce_sum(sm[:, :], pt[:, :],
                                             axis=mybir.AxisListType.X)
                        rs = pw.tile([128, 1], F32, tag="rs", bufs=2,
                                     name=f"rs_{g}_{bt}")
                        nc.vector.reciprocal(rs[:, :], sm[:, :])
                        nc.vector.tensor_scalar_mul(gsb[g][:, bt, :],
                                                    pt[:, :], rs[:, :])

                # ---- Phase B: experts, fc2 software-pipelined ---------
                acc = {h: pw.tile([128, BT, O], F32, tag=f"acc{h}", bufs=2,
                                  name=f"acc_{h}") for h in HEADS}
                first_seen = set()
                expert_state = {}

                def emit_mm2(g, e, ht, hT, w2t, ht4):
                    if ht == 0:
                        expert_state[(g, e)]["po"] = [
                            ps.tile([128, O], F32, tag=f"po{bt}", bufs=1,
                                    name=f"po_{g}_{e}_{bt}")
                            for bt in range(BT)]
                    st = expert_state[(g, e)]
                    psum_o = st["po"]
                    for bt in range(BT):
                        nc.tensor.matmul(
                            psum_o[bt][:, :],
                            hT[:, bt * 128:(bt + 1) * 128],
                            w2t[:, ht4, :],
                            start=(ht == 0), stop=(ht == HT - 1))
                    if ht != HT - 1:
                        return
                    # expert tail: drain + b2 + gated accum, all DVE.
                    # (Pool/gpsimd cannot read PSUM nor run TensorScalarPtr
                    # on core-v3, so it cannot take any of this; the relu
                    # epilogues alternate DVE/ACT so half of them dodge the
                    # tail burst anyway.)
                    eng = nc.vector
                    b2r = st["b2r"]
                    for bt in range(BT):
                        o_sb = pw.tile([128, O], F32, tag="o_sb", bufs=4,
                                       name=f"osb_{g}{e}_{bt}")
                        nc.vector.tensor_add(o_sb[:, :], psum_o[bt][:, :],
                                             b2r[:, :])
                        for head, gate, col in _contribs(g, e):
                            gcol = gsb[gate][:, bt, col:col + 1]
                            if (head, bt) not in first_seen:
                                eng.tensor_scalar_mul(
                                    acc[head][:, bt, :], o_sb[:, :], gcol)
                                first_seen.add((head, bt))
                            else:
                                eng.scalar_tensor_tensor(
                                    acc[head][:, bt, :], o_sb[:, :],
                                    gcol, acc[head][:, bt, :],
                                    op0=mybir.AluOpType.mult,
                                    op1=mybir.AluOpType.add)

                pending = []
                SKEW = 2
                step = 0
                for g in GROUPS:
                    for e in range(E):
                        b1_sb = pw.tile([128, HT], F32, tag="b1", bufs=2,
                                        name=f"b1_{g}{e}")
                        nc.sync.dma_start(b1_sb[:, :], b1[g][e, :, :])
                        b2_sb = pw.tile([1, O], DT, tag="b2", bufs=2,
                                        name=f"b2_{g}{e}")
                        nc.sync.dma_start(b2_sb[:, :], b2[g][e][None, :])
                        if mode != "dma":
                            # b2 replicated across partitions: ones.T @ b2
                            pb = ps.tile([128, O], F32, tag="ph", bufs=4,
                                         name=f"pb_{g}{e}")
                            nc.tensor.matmul(pb[:, :], ones[:, :],
                                             b2_sb[:, :], start=True,
                                             stop=True)
                            b2r = pw.tile([128, O], F32, tag="b2r", bufs=2,
                                          name=f"b2r_{g}{e}")
                            nc.scalar.copy(b2r[:, :], pb[:, :])
                            expert_state[(g, e)] = {"b2r": b2r}
                        for ht in range(HT):
                            htg, ht4 = divmod(ht, HTG)
                            if ht4 == 0:
                                w1t = pw.tile([128, IT, 512], DT, tag="w1",
                                              bufs=4, name=f"w1_{g}{e}_{htg}")
                                w2t = pw.tile([128, HTG, O], DT, tag="w2",
                                              bufs=4, name=f"w2_{g}{e}_{htg}")
                                if mode != "compute":
                                    nc.sync.dma_start(w1t[:, :, :],
                                                      w1[g][e, htg])
                                    nc.sync.dma_start(w2t[:, :, :],
                                                      w2[g][e, htg])
                                else:
                                    nc.sync.dma_start(
                                        w1t[:, 0, 0:1],
                                        w1[g][e, htg, :, 0, 0:1])
                                    nc.sync.dma_start(
                                        w2t[:, 0, 0:1],
                                        w2[g][e, htg, :, 0, 0:1])

                            if mode == "dma":
                                continue
                            ph = ps.tile([128, BL], F32, tag="ph", bufs=4,
                                         name=f"ph_{g}{e}_{ht}")
                            for it in range(IT):
                                nc.tensor.matmul(
                                    ph[:, :],
                                    w1t[:, it, ht4 * 128:(ht4 + 1) * 128],
                                    xTs[g][:, it, :],
                                    start=(it == 0),
                                    stop=(it == IT - 1))
                            hT = pw.tile([128, BL], DT, tag="hT", bufs=6,
                                         name=f"hT_{g}{e}_{ht}")
                            # relu(ph + b1); alternate DVE/ACT to split the
                            # epilogue load across both engines
                            if step % 2 == 0:
                                nc.vector.tensor_scalar(
                                    hT[:, :], ph[:, :],
                                    b1_sb[:, ht:ht + 1], 0.0,
                                    op0=mybir.AluOpType.add,
                                    op1=mybir.AluOpType.max)
                            else:
                                nc.scalar.activation(
                                    hT[:, :], ph[:, :],
                                    mybir.ActivationFunctionType.Relu,
                                    bias=b1_sb[:, ht:ht + 1], scale=1.0)
                            pending.append((g, e, ht, hT, w2t, ht4))
                            if len(pending) > SKEW:
                                emit_mm2(*pending.pop(0))
                            step += 1
                while pending:
                    emit_mm2(*pending.pop(0))

                # ---- store outputs -----------------------------------
                for h in (() if mode == "dma" else HEADS):
                    for bt in range(BT):
                        nc.sync.dma_start(outs[h][bt * 128:(bt + 1) * 128, :],
                                          acc[h][:, bt, :])

    nc.finalize()
    return nc


def prep_inputs(inputs, mm="bf16"):
    """Host-side layout/dtype prep. Returns (shared weight map,
    per-core x maps); in_maps[c] = {**shared, **xmaps[c]}."""
    import ml_dtypes
    mdt = ml_dtypes.bfloat16 if mm == "bf16" else np.float32
    f = {k: np.asarray(v, dtype=np.float32) for k, v in inputs.items()}

    shared = {}
    for g in GROUPS:
        W1 = f[f"w1_{g}"]          # [E, I, H]
        shared[f"w1_{g}"] = np.ascontiguousarray(
            W1.reshape(E, IT, 128, NHTG, 512).transpose(0, 3, 2, 1, 4)
        ).astype(mdt)
        W2 = f[f"w2_{g}"]          # [E, H, O]
        shared[f"w2_{g}"] = np.ascontiguousarray(
            W2.reshape(E, NHTG, HTG, 128, O).transpose(0, 1, 3, 2, 4)
        ).astype(mdt)
        shared[f"b1_{g}"] = np.ascontiguousarray(
            f[f"b1_{g}"].reshape(E, HT, 128).transpose(0, 2, 1))
        shared[f"b2_{g}"] = f[f"b2_{g}"].astype(mdt)
        WG = f[f"wg_{g}"]          # [I, GW]
        shared[f"wg_{g}"] = np.ascontiguousarray(
            WG.reshape(IT, 128, GATE_W[g]).transpose(1, 0, 2)).astype(mdt)
        shared[f"bg_{g}"] = f[f"bg_{g}"]

    xs = {"sh": f["x_shared"], "t1": f["x_task1"], "t2": f["x_task2"]}
    xmaps = []
    for c in range(N_CORES):
        sl = slice(c * BL, (c + 1) * BL)
        m = {}
        for g in GROUPS:
            xt = xs[g][sl].T                       # [I, BL]
            m[f"xT_{g}"] = np.ascontiguousarray(
                xt.reshape(IT, 128, BL).transpose(1, 0, 2)).astype(mdt)
        xmaps.append(m)
    return shared, xmaps


_NC_CACHE = None


def _get_nc():
    global _NC_CACHE
    if _NC_CACHE is None:
        _NC_CACHE = build_nc()
    return _NC_CACHE


def kernel(**inputs) -> tuple:
    from concourse.bass_utils import run_bass_kernel_spmd

    nc = _get_nc()
    shared, xmaps = prep_inputs(inputs)
    in_maps = [{**shared, **xmaps[c]} for c in range(N_CORES)]

    # rare transient NRT_EXEC_UNIT_UNRECOVERABLE crashes have been observed
    # on this fabric; retry a couple of times before giving up
    last_err = None
    for attempt in range(3):
        try:
            r = run_bass_kernel_spmd(nc, in_maps, list(range(N_CORES)))
            break
        except Exception as ex:  # noqa: BLE001
            last_err = ex
            import time as _time
            _time.sleep(5 * (attempt + 1))
    else:
        raise last_err
    out_sh = np.concatenate([r.results[c]["osh"] for c in range(N_CORES)], axis=0)
    out1 = np.concatenate([r.results[c]["o1"] for c in range(N_CORES)], axis=0)
    out2 = np.concatenate([r.results[c]["o2"] for c in range(N_CORES)], axis=0)
    return (out_sh, out1, out2)
